# revision 2
# baseline (speedup 1.0000x reference)
"""Position-attention kernel for Trainium2 (8 NeuronCores, SPMD).

Reference computation (per batch b):
    q = Wq @ x + bq        [32, 4096]
    k = Wk @ x + bk        [32, 4096]
    v = Wv @ x + bv        [256, 4096]
    attn = softmax_j(q_i . k_j)           [4096, 4096]
    out[c, i] = sum_j v[c, j] attn[i, j]
    y = gamma * out + x

Sharding: B=4 batches x 2 query-halves -> 8 cores. Each core computes the
full softmax rows for its 2048 queries against all 4096 keys of its batch.
Host rotates x columns per core so the core's query half is always columns
0:2048 (softmax and the PV contraction are invariant to key/value column
order, as long as K and V use the same order).

Device-side structure (per core):
  - projections in bf16 (x pre-cast on host; weights pre-packed on host).
  - K projection col-tiled: stationary WkT [128, 32] at tile_position
    (0, 32*(j%4)) so the four 32-row outputs of a quad of key blocks land
    packed in one PSUM tile; kf stored [32*(j%4)+d, j//4, :].
  - scores computed transposed (sT[j, i]) in PSUM, 3 key-blocks at a time
    packed into PE row-groups 32*(j%4) via tile_position (the K=32
    contractions run concurrently in the array); q replicated into all
    four row groups.
  - exp is split across two engines: ACT does the leading groups
    (exact exp), DVE does the trailing groups of each supertile with a
    Schraudolph fast-exp (one tensor_scalar: int16 bits = round(s*A + B),
    bitcast to bf16; +-3% sawtooth that largely cancels in the softmax
    ratio). This removes ACT as the steady-state co-bottleneck.
  - PV: out[i, c] = sum_j e[j, i] * vT[j, c] with e-blocks as the stationary
    operand; vT carries an extra all-ones column so column 256 of the
    output is the softmax denominator (per-partition = per-query).
  - epilogue: y_T[i, :] = out[i, :] * (gamma / sum_i) + (x_T[i, :] +
    gamma * bv)  -- the bv term works because sum_j attn = 1; it is folded
    into the precomputed xpb tile. Output written transposed; host
    transposes back (pure layout).
  - a warmup burst of dense matmuls at t=0 tries to trip the PE HAM
    activity monitor to full clock before real work.
"""

import os
import numpy as np

P = 128
B = 4
C = 256
CQ = 32
H = W = 64
N = H * W            # 4096 keys per batch
NH = N // 2          # 2048 queries per core
NCB = C // P         # 2 channel blocks
ST = 512             # query supertile
NST = NH // ST       # 4
JB = N // P          # 32 key blocks
NQG = JB // 4        # 8 k-proj quad groups
# score groups: 10 triples + 1 pair of key blocks (3-bank PSUM tiles allow
# double buffering: 2*3 + 2 out banks = 8)
GROUPS = [list(range(3 * g, min(3 * g + 3, 32))) for g in range(11)]
NGR = len(GROUPS)
# exp engine split: first ACT_N groups of a supertile go to ACT (exact exp),
# the rest to DVE (Schraudolph). st0 gives ACT one more (DVE is busy with
# projections early on).
ACT_N = {0: 8, 1: 7, 2: 7, 3: 7}
NWARM = 30           # dense warmup matmuls to trip the HAM to full clock

# Schraudolph fast-exp constants: bf16 bits = round(s * SCHRA_A + SCHRA_B)
SCHRA_A = 184.66496523378732          # 128 * log2(e)
SCHRA_B = 16250.515                    # 127*128 - 128*0.0602/(2 ln2): centered

_PROG = None         # cached build
LAST_RESULT = None   # BassKernelResults of the last run (for test harness)


def _build_program():
    import concourse.mybir as mybir
    import concourse.tile as tile
    from concourse import bacc
    from concourse.bass import ds

    fp32 = mybir.dt.float32
    bf16 = mybir.dt.bfloat16
    i16 = mybir.dt.int16

    nc = bacc.Bacc(None, target_bir_lowering=False, debug=False)

    xb_d = nc.declare_dram_parameter("xb", [C, N], bf16, isOutput=False)
    # xpb = xT + gamma*bv, already in SBUF layout [p, row_block*C]
    xpb_d = nc.declare_dram_parameter("xpb", [P, (NH // P) * C], fp32, isOutput=False)
    wq_d = nc.declare_dram_parameter("wq_pre", [P, NCB * P], bf16, isOutput=False)
    wk_d = nc.declare_dram_parameter("wkT_pre", [P, NCB * CQ], bf16, isOutput=False)
    wv_d = nc.declare_dram_parameter("wv_pre", [P, NCB * C], bf16, isOutput=False)
    bq_d = nc.declare_dram_parameter("bq_rep", [P, 1], fp32, isOutput=False)
    bk_d = nc.declare_dram_parameter("bk_pack", [P, 1], fp32, isOutput=False)
    gm_d = nc.declare_dram_parameter("gamma_bc", [P, 1], fp32, isOutput=False)
    yT_d = nc.declare_dram_parameter("yT", [NH, C], fp32, isOutput=True)

    with tile.TileContext(nc) as tc:
        with (
            tc.tile_pool(name="singles", bufs=1) as singles,
            tc.tile_pool(name="epool", bufs=24) as epool,
            tc.tile_pool(name="stpool", bufs=4) as stpool,
            tc.tile_pool(name="ivpool", bufs=4) as ivpool,
            tc.tile_pool(name="pp_mm", bufs=2, space="PSUM") as pp_mm,
            tc.tile_pool(name="pp_out", bufs=2, space="PSUM") as pp_out,
        ):
            # ---- persistent SBUF tensors ----
            xb_sb = singles.tile([P, NCB, N], bf16)
            xpb_sb = singles.tile([P, NH // P, C], fp32)  # xT + gamma*bv
            wq_sb = singles.tile([P, NCB, P], bf16)
            wkT_sb = singles.tile([P, NCB, CQ], bf16)
            wv_sb = singles.tile([P, NCB, C], bf16)
            bq_sb = singles.tile([P, 1], fp32)
            bk_sb = singles.tile([P, 1], fp32)
            gm_sb = singles.tile([P, 1], fp32)
            kf_sb = singles.tile([P, NQG, P], bf16)  # row 32*(j%4)+d, quad j//4
            q_sb = singles.tile([P, NH], bf16)       # q replicated in 4 groups
            vT_sb = singles.tile([P, JB, C + 1], bf16)  # col C is all-ones

            # ---- boot-time warmup, before any DMA-dependent work ----
            # dense back-to-back matmuls to push the PE HAM activity monitor
            # over its busy threshold so real matmuls run at 2.4 GHz, plus a
            # dummy exp to pre-load the ACT function table (~2.7us).
            warm_sb = singles.tile([P, P], bf16)
            warm_e = singles.tile([1, 1], fp32)
            nc.vector.memset(warm_sb[:], 0.0)
            nc.scalar.activation(
                warm_e, warm_sb[0:1, 0:1], mybir.ActivationFunctionType.Exp
            )
            for w in range(NWARM):
                wp = pp_out.tile([P, P], fp32, tag="out", name=f"warm_{w}")
                nc.tensor.matmul(wp, warm_sb, warm_sb, start=True, stop=True)

            # ---- input DMAs. Per-queue BW is ~34 GB/s, so spread the big
            # tensors over many queues with >=2KB descriptors. wk first (the
            # K projection needs it), then x column-chunks in consumption
            # order (each 512-col chunk is exactly one k-proj quad group),
            # interleaved with the remaining weights. xpb (residual) goes
            # LAST: first use is the st0/ib0 epilogue, ~10us in.
            nc.sync.dma_start(
                out=wkT_sb[:], in_=wk_d.rearrange("p (o m) -> p o m", o=NCB)
            )
            col_chunks = [(c0, 512) for c0 in range(0, N, 512)]
            for c0, cw in col_chunks:
                csl = ds(c0, cw)
                for cb in range(NCB):
                    for rh in range(2):
                        nc.sync.dma_start(
                            out=xb_sb[rh * 64:(rh + 1) * 64, cb, csl],
                            in_=xb_d[cb * P + rh * 64:cb * P + (rh + 1) * 64, csl],
                        )
                if c0 == 0:
                    nc.sync.dma_start(
                        out=wq_sb[:], in_=wq_d.rearrange("p (o m) -> p o m", o=NCB)
                    )
                    nc.sync.dma_start(out=bq_sb[:], in_=bq_d[:])
                    nc.sync.dma_start(out=bk_sb[:], in_=bk_d[:])
                    nc.sync.dma_start(out=gm_sb[:], in_=gm_d[:])
                elif c0 == 1024:
                    for rh in range(2):
                        rsl = slice(rh * 64, (rh + 1) * 64)
                        nc.sync.dma_start(
                            out=wv_sb[rsl],
                            in_=wv_d[rsl].rearrange("p (o m) -> p o m", o=NCB),
                        )
            xpb_flat = xpb_sb.rearrange("p o c -> p (o c)")
            for rh in range(2):
                for cc in range(8):
                    rsl = slice(rh * 64, (rh + 1) * 64)
                    csl = ds(cc * 512, 512)
                    nc.sync.dma_start(
                        out=xpb_flat[rsl, csl], in_=xpb_d[rsl, csl]
                    )

            nc.vector.memset(vT_sb[:, :, C:C + 1], 1.0)

            # ---- K projection, col-tiled: quad qg covers key blocks
            # 4qg+r; stationary is the plain 32-col WkT so the four 32-row
            # outputs run concurrently in separate column groups.
            def k_proj(qg):
                kp = pp_out.tile([P, P], fp32, tag="out", name=f"kp_{qg}")
                for r in range(4):
                    j = 4 * qg + r
                    for cb in range(NCB):
                        nc.tensor.matmul(
                            kp[32 * r:32 * r + 32, :],
                            wkT_sb[:, cb, :],
                            xb_sb[:, cb, ds(j * P, P)],
                            start=(cb == 0), stop=(cb == NCB - 1),
                            tile_position=(0, 32 * r),
                        )
                nc.vector.tensor_scalar_add(kf_sb[:, qg, :], kp, bk_sb)

            def v_proj_pair(t):
                # two key-blocks per PSUM tile (exactly one bank) and one
                # PSUM->SBUF cast; first half goes to ACT (idle in the head
                # while exp hasn't started), second half to DVE.
                vp = pp_out.tile([P, 2, C], fp32, tag="out", name=f"vp_{t}")
                for u in range(2):
                    j = 2 * t + u
                    nc.tensor.matmul(
                        vp[:, u], xb_sb[:, 0, ds(j * P, P)], wv_sb[:, 0],
                        start=True, stop=False,
                    )
                    nc.tensor.matmul(
                        vp[:, u], xb_sb[:, 1, ds(j * P, P)], wv_sb[:, 1],
                        start=False, stop=True,
                    )
                if t < 8:
                    nc.scalar.activation(
                        vT_sb[:, 2 * t:2 * t + 2, 0:C], vp,
                        mybir.ActivationFunctionType.Copy,
                    )
                else:
                    nc.vector.tensor_copy(vT_sb[:, 2 * t:2 * t + 2, 0:C], vp)

            # ---- Q projection chunk (replicated across the 4 row groups) ----
            def q_proj(t):
                qp = pp_out.tile([P, ST], fp32, tag="out", name=f"qp_{t}")
                nc.tensor.matmul(
                    qp, wq_sb[:, 0], xb_sb[:, 0, ds(t * ST, ST)],
                    start=True, stop=False,
                )
                nc.tensor.matmul(
                    qp, wq_sb[:, 1], xb_sb[:, 1, ds(t * ST, ST)],
                    start=False, stop=True,
                )
                nc.vector.tensor_scalar_add(q_sb[:, ds(t * ST, ST)], qp, bq_sb)

            es_by_st = [[] for _ in range(NST)]
            vp_cnt = [0]

            def scores_group(st_i, g):
                js = GROUPS[g]
                nr = len(js)
                sps = pp_mm.tile([P, 3, ST], fp32, tag="mm", name=f"sps_{st_i}_{g}")
                for r, j in enumerate(js):
                    m = j % 4
                    nc.tensor.matmul(
                        sps[:, r],
                        kf_sb[32 * m:32 * (m + 1), j // 4, :],
                        q_sb[32 * m:32 * (m + 1), ds(st_i * ST, ST)],
                        start=True, stop=True,
                        tile_position=(32 * m, 0),
                    )
                e = epool.tile([P, 3, ST], bf16, name=f"e_{st_i}_{g}", tag="e")
                if g < ACT_N[st_i]:
                    nc.scalar.activation(
                        e[:, 0:nr], sps[:, 0:nr], mybir.ActivationFunctionType.Exp
                    )
                else:
                    nc.vector.tensor_scalar(
                        e[:, 0:nr].bitcast(i16), sps[:, 0:nr],
                        SCHRA_A, SCHRA_B,
                        mybir.AluOpType.mult, mybir.AluOpType.add,
                    )
                es_by_st[st_i].append(e)

            # ---- st0 score/exp phase: fill the PE with the K/Q projections
            # (just-in-time per quad) and the V projection.
            k_proj(0)
            q_proj(0)
            kq_done = 1

            for g in range(NGR):
                if g + 1 < NGR:
                    need = (3 * (g + 1) + 2) // 4
                    while kq_done <= min(need, NQG - 1):
                        k_proj(kq_done)
                        kq_done += 1
                if g in (2, 5, 8):
                    q_proj({2: 1, 5: 2, 8: 3}[g])
                scores_group(0, g)
                while (vp_cnt[0] + 1) * 2 <= 3 * (g + 1) and vp_cnt[0] < JB // 2:
                    v_proj_pair(vp_cnt[0])
                    vp_cnt[0] += 1

            # ---- PV phases; scores/exp of the NEXT supertile are woven in
            # so ACT/DVE work during PV instead of pacing afterwards.
            for st_i in range(NST):
                es = es_by_st[st_i]
                nxt = 0
                cnt = 0
                for ib in range(4):
                    out_ps = pp_out.tile(
                        [P, C + 1], fp32, tag="out", name=f"out_{st_i}_{ib}"
                    )
                    for j in range(JB):
                        nc.tensor.matmul(
                            out_ps,
                            es[min(j // 3, 10)][:, j - 3 * min(j // 3, 10), ds(ib * P, P)],
                            vT_sb[:, j, :],
                            start=(j == 0), stop=(j == JB - 1),
                        )
                        cnt += 1
                        if cnt % 10 == 0 and st_i + 1 < NST and nxt < NGR:
                            scores_group(st_i + 1, nxt)
                            nxt += 1
                    # epilogue: per-partition normalize + gamma + residual
                    row = st_i * 4 + ib
                    inv = ivpool.tile([P, 1], fp32)
                    nc.vector.reciprocal(inv, out_ps[:, C:C + 1])
                    nc.vector.tensor_scalar_mul(inv, inv, gm_sb)
                    stg = stpool.tile([P, C], fp32)
                    nc.vector.scalar_tensor_tensor(
                        stg, out_ps[:, 0:C], inv, xpb_sb[:, row, :],
                        op0=mybir.AluOpType.mult,
                        op1=mybir.AluOpType.add,
                    )
                    # split across two queues to cut the store latency
                    for rq in range(2):
                        nc.sync.dma_start(
                            out=yT_d[ds(row * P + rq * 64, 64), :],
                            in_=stg[rq * 64:(rq + 1) * 64, :],
                        )
                while st_i + 1 < NST and nxt < NGR:
                    scores_group(st_i + 1, nxt)
                    nxt += 1

    return nc


def _get_program():
    global _PROG
    if _PROG is None:
        _PROG = _build_program()
        if not _PROG.is_finalized():
            _PROG.finalize()
    return _PROG


def kernel(x, Wq, bq, Wk, bk, Wv, bv, gamma):
    global LAST_RESULT
    import ml_dtypes
    from concourse.bass_utils import run_bass_kernel_spmd

    bf16 = ml_dtypes.bfloat16
    x = np.ascontiguousarray(np.asarray(x, dtype=np.float32))
    Wq = np.asarray(Wq, dtype=np.float32)
    bq = np.asarray(bq, dtype=np.float32)
    Wk = np.asarray(Wk, dtype=np.float32)
    bk = np.asarray(bk, dtype=np.float32)
    Wv = np.asarray(Wv, dtype=np.float32)
    bv = np.asarray(bv, dtype=np.float32)
    gamma = np.asarray(gamma, dtype=np.float32)

    # wq replicated into all four 32-row groups of the PE array
    wq_rep = np.zeros((C, P), dtype=np.float32)
    for r in range(4):
        wq_rep[:, 32 * r:32 * (r + 1)] = Wq.T
    bq_rep = np.tile(bq, 4)[:, None].astype(np.float32)
    bk_pack = np.tile(bk, 4)[:, None].astype(np.float32)
    gval = float(gamma.reshape(-1)[0])
    gm_bc = np.full((P, 1), gval, dtype=np.float32)

    def _swz(a):
        # [C, F] -> [128, NCB*F]: exact SBUF layout (partition-major)
        f = a.reshape(NCB, P, -1)
        return np.ascontiguousarray(
            f.transpose(1, 0, 2).reshape(P, -1).astype(bf16)
        )

    wq_pre = _swz(wq_rep)
    wkT_pre = _swz(Wk.T)
    wv_pre = _swz(Wv.T)

    xf = x.reshape(B, C, N)
    in_maps = []
    for core in range(8):
        b, h = core // 2, core % 2
        xb = xf[b]
        if h == 0:
            x_roll = xb
        else:
            x_roll = np.concatenate([xb[:, NH:], xb[:, :NH]], axis=1)
        # xpb[p, o, c] = x_roll[c, o*128 + p] + gamma*bv[c]  (SBUF layout)
        xqT = x_roll[:, :NH].T + gval * bv[None, :]
        xpb = np.ascontiguousarray(
            xqT.reshape(NH // P, P, C).transpose(1, 0, 2).reshape(P, (NH // P) * C)
        ).astype(np.float32)
        in_maps.append({
            "xb": np.ascontiguousarray(x_roll.astype(bf16)),
            "xpb": xpb,
            "wq_pre": wq_pre,
            "wkT_pre": wkT_pre,
            "wv_pre": wv_pre,
            "bq_rep": bq_rep,
            "bk_pack": bk_pack,
            "gamma_bc": gm_bc,
        })

    nc = _get_program()
    res = run_bass_kernel_spmd(
        nc, in_maps, core_ids=list(range(8)),
        trace=bool(os.environ.get("BASS_TRACE")),
    )
    LAST_RESULT = res

    out = np.empty((B, C, N), dtype=np.float32)
    for core in range(8):
        b, h = core // 2, core % 2
        yT = res.results[core]["yT"]
        out[b][:, h * NH:(h + 1) * NH] = yT.T
    return out.reshape(B, C, H, W)
